# revision 52
# baseline (speedup 1.0000x reference)
"""AgentAwareAttention Trainium2 kernel.

Full (unsharded) inputs -> full output.  Internally: 16 (batch, head) pairs
sharded 2-per-core across 8 NeuronCores; host pre-transposes operands and
sorts the sequence by agent id so the agent-equality mask becomes
block-diagonal.

Device-side structure (per core, heads h0/h1 of one batch):
  - projections:  QT = [q|qs] (bf16), QM = [-q|qs], KT = [k|ks]
  - main scores:  KT[k].T @ QT[q]           (32-contract, bf16)
  - delta scores: mask(KT)[k;ks].T @ QM[-q;qs]  adds (qs.ks - q.k) on the
    block-diagonal rects; masks are zero-padded KT column tiles built on the
    idle Pool engine (zeroed via KT*0 so the zeroing DEPENDS on KT and the
    scheduler cannot hoist every mask ahead of the first dep-ready copy).
  - exp on the Act engine is the bottleneck (~66us of [128,1024] tiles);
    PV uses ones-augmented V (rowsum for free); normalize + merged-head
    64-contract out-proj; j-pairs share one psum bank / osb / output DMA.

All four projection chunks are emitted BEFORE any score unit that reads
them (every S(u) reads all 16 KT key tiles -- interleaving proj(lc) behind
S(u) is a read-before-write race that CoreSim catches).  Emission is
software-pipelined over units u=(lc,h): S(u)=scores+exp, P(u)=PV,
O(lc)=normalize+outproj, ordered so the in-order PE stream never blocks
the Act engine's exp stream:
  p0 S0a p1 S0b p2 S0c p3 S0d S1 V P0 S2 P1 S3 O0 P2 S4 P3 S5 O1 P4 S6 P5 S7 O2 P6 P7 O3

Shapes (hardcoded): L=2048, N=2, E=256, H=8, D=32, N_AGENTS=16.
"""

import os

import numpy as np
from ml_dtypes import bfloat16


L = 2048
NB = 2        # batch
E = 256       # embed dim
H = 8         # heads
D = 32        # head dim
NCORES = 8
LC = 512      # l-chunk (moving-operand free dim)
NT = L // 128   # 16 s'-tiles of 128
NLC = L // LC   # 4 l-chunks

_PROGRAM_CACHE = {}


def _block_structure(ids):
    """Sort positions by agent id.  Returns perm and per-agent ranges in
    permuted space."""
    ids = np.asarray(ids)
    perm = np.argsort(ids, kind="stable")
    sids = ids[perm]
    bounds = [0]
    for i in range(1, len(sids)):
        if sids[i] != sids[i - 1]:
            bounds.append(i)
    bounds.append(len(sids))
    blocks = [(bounds[i], bounds[i + 1]) for i in range(len(bounds) - 1)]
    return perm, blocks


def _rects(blocks):
    """rects[(t, lc)] -> list of (r0, r1, c0, c1): the part of diagonal block
    (rows x cols, both = the block's range) that intersects s'-tile t
    (rows [128t,128t+128)) and l-chunk lc (cols [LC*lc, LC*lc+LC)), in
    tile-local coordinates."""
    rects = {}
    for (b0, b1) in blocks:
        for t in range(NT):
            r0 = max(b0, 128 * t)
            r1 = min(b1, 128 * t + 128)
            if r0 >= r1:
                continue
            for lc in range(NLC):
                c0 = max(b0, LC * lc)
                c1 = min(b1, LC * lc + LC)
                if c0 >= c1:
                    continue
                rects.setdefault((t, lc), []).append(
                    (r0 - 128 * t, r1 - 128 * t, c0 - LC * lc, c1 - LC * lc)
                )
    return rects


def _build_program(rects):
    import concourse.mybir as mybir
    import concourse.tile as tile
    from concourse import bacc

    f32 = mybir.dt.float32
    bf16 = mybir.dt.bfloat16
    add = mybir.AluOpType.add
    mult = mybir.AluOpType.mult

    # masks needed: (t, r0, r1) -> arena index; full-tile rects use KT directly
    mask_idx = {}
    for (t, lc), rl in sorted(rects.items()):
        for (r0, r1, c0, c1) in rl:
            if (r0, r1) == (0, 128):
                continue
            key = (t, r0, r1)
            if key not in mask_idx:
                mask_idx[key] = len(mask_idx)
    n_masks = max(1, len(mask_idx))

    nc = bacc.Bacc(None)

    xT_d = nc.declare_dram_parameter("xT", [E, L], bf16, isOutput=False)
    # wpack: [wqkv half0 (320) | wqkv half1 (320) | wo (256)]
    wpack_d = nc.declare_dram_parameter("wpack", [128, 896], bf16, isOutput=False)
    bias4_d = nc.declare_dram_parameter("bias4", [128, 4], f32, isOutput=False)
    out_d = nc.declare_dram_parameter("out", [L, E], f32, isOutput=True)

    with tile.TileContext(nc) as tc:
        with (
            tc.tile_pool(name="consts", bufs=1) as consts,
            tc.tile_pool(name="pslab", bufs=3) as pslab_pool,
            tc.tile_pool(name="onorm", bufs=4) as onorm_pool,
            tc.tile_pool(name="small", bufs=4) as small_pool,
            tc.tile_pool(name="outsb", bufs=3) as outsb_pool,
            tc.tile_pool(name="ps_score", bufs=2, space="PSUM") as ps_score,
            tc.tile_pool(name="ps_self", bufs=2, space="PSUM") as ps_self,
            tc.tile_pool(name="ps_oacc", bufs=2, space="PSUM") as ps_oacc,
        ):
            # ---- constant loads -------------------------------------------
            # HWDGE serializes DMA issue at ~625ns each, so everything
            # constant rides ONE packed DMA; xt is chunked so proj(0) can
            # start as soon as its first 512 columns land.
            xt = [consts.tile([128, L], bf16, tag=f"xt{i}", name=f"xt{i}")
                  for i in range(2)]
            wpack = consts.tile([128, 896], bf16, tag="wpack", name="wpack")
            bias4 = consts.tile([128, 4], f32, tag="bias4", name="bias4")
            wq = [wpack[:, 320 * i:320 * i + 128] for i in range(2)]
            wk = [wpack[:, 320 * i + 128:320 * i + 256] for i in range(2)]
            wv = [wpack[:, 320 * i + 256:320 * i + 320] for i in range(2)]
            bq_t = bias4[:, 0:1]
            bk_t = bias4[:, 1:2]
            bqm_t = bias4[:, 2:3]
            sg_t = bias4[:, 3:4]
            wo_pair = wpack[0:64, 640:896]
            # wpack first (weights gate the first matmul), then lc0 xt
            nc.sync.dma_start(wpack, wpack_d[:, :])
            for i in range(2):
                nc.sync.dma_start(xt[i][:, 0:LC],
                                  xT_d[128 * i:128 * (i + 1), 0:LC])
            nc.sync.dma_start(bias4, bias4_d[:, :])
            for i in range(2):
                nc.sync.dma_start(xt[i][:, LC:2 * LC],
                                  xT_d[128 * i:128 * (i + 1), LC:2 * LC])
            for i in range(2):
                nc.sync.dma_start(xt[i][:, 2 * LC:],
                                  xT_d[128 * i:128 * (i + 1), 2 * LC:])

            QT = consts.tile([128, L], bf16, tag="QT", name="QT")
            QM = consts.tile([128, L], bf16, tag="QM", name="QM")
            KT = consts.tile([128, L], bf16, tag="KT", name="KT")
            marena = consts.tile([128, n_masks, 128], bf16, tag="marena",
                                 name="marena")
            v_sb = consts.tile([128, NT, 66], bf16, tag="vsb", name="v_sb")

            # PE warm-up: TRN2 pstate ramps (0.65 -> 1.2 -> 2.4 GHz) key off
            # continuous PE activity; a few throwaway matmuls on a memset
            # tile keep the projection matmuls off the cold clock.
            dummy = consts.tile([128, 512], bf16, tag="dummy", name="dummy")
            dume = consts.tile([1, 1], bf16, tag="dume", name="dume")
            nc.gpsimd.memset(dummy, 0.0)
            nc.gpsimd.memset(dume, 0.0)
            nc.gpsimd.memset(v_sb, 1.0)
            # hoist the Exp act-table load off the critical path
            nc.scalar.activation(dume, dummy[0:1, 0:1],
                                 mybir.ActivationFunctionType.Exp)
            for w in range(2):
                pdum = ps_self.tile([128, 512], f32, tag="self",
                                    name="ps_self_t")
                nc.tensor.matmul(pdum, dummy[:, 0:128], dummy,
                                 start=True, stop=True)

            def delta_lhs(t, r0, r1):
                if (r0, r1) == (0, 128):
                    return KT[:, 128 * t:128 * (t + 1)]
                m = mask_idx[(t, r0, r1)]
                return marena[:, m, :]

            def proj(lc):
                sl = slice(LC * lc, LC * (lc + 1))
                pk = ps_self.tile([128, 512], f32, tag="self", name="ps_self_t")
                nc.tensor.matmul(pk, wk[0], xt[0][:, sl], start=True, stop=False)
                nc.tensor.matmul(pk, wk[1], xt[1][:, sl], start=False, stop=True)
                nc.vector.tensor_scalar(
                    out=KT[:, sl], in0=pk, scalar1=bk_t, scalar2=None, op0=add)
                # mask tiles sourced from this KT chunk (idle Pool engine).
                # zeroing via KT*0 (not memset) so it DEPENDS on this KT
                # chunk -- otherwise the scheduler hoists every no-dep memset
                # ahead of the first dep-ready copy and starves lc0's masks.
                for (t, r0, r1), m in sorted(mask_idx.items(), key=lambda kv: kv[1]):
                    if t // 4 == lc:
                        nc.gpsimd.tensor_scalar(
                            out=marena[:, m, :],
                            in0=KT[:, 128 * t:128 * (t + 1)],
                            scalar1=0.0, scalar2=None, op0=mult)
                        nc.gpsimd.tensor_copy(
                            marena[:, m, r0:r1],
                            KT[:, 128 * t + r0:128 * t + r1])
                pq = ps_self.tile([128, 512], f32, tag="self", name="ps_self_t")
                nc.tensor.matmul(pq, wq[0], xt[0][:, sl], start=True, stop=False)
                nc.tensor.matmul(pq, wq[1], xt[1][:, sl], start=False, stop=True)
                nc.vector.tensor_scalar(
                    out=QT[:, sl], in0=pq, scalar1=bq_t, scalar2=None, op0=add)
                if lc == 0:
                    # Act idles pre-exp; DVE's scheduler otherwise orders KT1
                    # ahead of QM0 and delays the first delta matmuls
                    nc.scalar.activation(
                        QM[:, sl], pq, mybir.ActivationFunctionType.Identity,
                        bias=bqm_t, scale=sg_t)
                else:
                    nc.vector.tensor_scalar(
                        out=QM[:, sl], in0=pq, scalar1=sg_t, scalar2=bqm_t,
                        op0=mult, op1=add)

            def vbuild():
                for t in range(NT):
                    ts = slice(128 * t, 128 * (t + 1))
                    pv = ps_self.tile([128, 512], f32, tag="self",
                                      name="ps_self_t")
                    nc.tensor.matmul(pv[:, 0:64], xt[0][:, ts], wv[0],
                                     start=True, stop=False)
                    nc.tensor.matmul(pv[:, 0:64], xt[1][:, ts], wv[1],
                                     start=False, stop=True)
                    nc.vector.tensor_copy(v_sb[:, t, 0:32], pv[:, 0:32])
                    nc.vector.tensor_copy(v_sb[:, t, 33:65], pv[:, 32:64])

            pslabs = {}
            oaccs = {}

            # score groups: contiguous t-runs of 3 (then the leftover t15)
            # -> 6 exps of width <=1536 per (lc,h) instead of 8x1024,
            # amortizing the Act engine's fixed per-instruction access cost
            SGROUPS = [[0, 1], [2, 3], [4, 5], [6, 7], [8, 9], [10, 11],
                       [12, 13], [14, 15]]
            # unit 0 leads with a narrow [t0] group: exp0 fires right after
            # QM0 lands instead of waiting for a full 1024-wide group
            SGROUPS0 = [[0], [1], [2, 3], [4, 5], [6, 7], [8, 9], [10, 11],
                        [12, 13], [14, 15]]

            def S(u, g0=0, g1=None):
                lc, h = u // 2, u % 2
                groups = SGROUPS0 if u == 0 else SGROUPS
                if g1 is None:
                    g1 = len(groups)
                lsl = slice(LC * lc, LC * (lc + 1))
                qb = 64 * h
                if g0 == 0:
                    pslab = pslab_pool.tile([128, NT * 512], bf16, tag="pslab",
                                            name="pslab")
                    pslabs[u] = pslab
                else:
                    pslab = pslabs[u]
                for g in range(g0, g1):
                    ts = groups[g]
                    w = 512 * len(ts)
                    ps2 = ps_score.tile([128, 1024], f32, tag="score",
                                        name="ps2")
                    for k, t in enumerate(ts):
                        tsl = slice(128 * t, 128 * (t + 1))
                        o = 512 * k
                        rl = rects.get((t, lc), [])
                        nc.tensor.matmul(
                            ps2[:, o:o + 512],
                            KT[qb:qb + 32, tsl], QT[qb:qb + 32, lsl],
                            start=True, stop=(not rl), tile_position=(qb, 0))
                        for i, (r0, r1, c0, c1) in enumerate(rl):
                            mk = delta_lhs(t, r0, r1)
                            nc.tensor.matmul(
                                ps2[:, o + c0:o + c1],
                                mk[qb:qb + 64, :],
                                QM[qb:qb + 64, LC * lc + c0:LC * lc + c1],
                                start=False, stop=(i == len(rl) - 1),
                                tile_position=(qb, 0))
                    nc.scalar.activation(
                        pslab[:, 512 * ts[0]:512 * ts[0] + w], ps2[:, 0:w],
                        mybir.ActivationFunctionType.Exp)

            def P(u):
                lc, h = u // 2, u % 2
                pslab = pslabs[u]
                oacc = ps_oacc.tile([33, 512], f32, tag="oacc", name="oacc")
                oaccs[u] = oacc
                for t in range(NT):
                    nc.tensor.matmul(
                        oacc, v_sb[:, t, 33 * h:33 * h + 33],
                        pslab[:, 512 * t:512 * (t + 1)],
                        start=(t == 0), stop=(t == NT - 1))

            def O(lc):
                # normalize both heads into one [64, 512] tile, then a single
                # 64-contract out-proj matmul per j-tile; j-pairs share one
                # psum bank / osb / output DMA.  The last chunk's copies ride
                # the (by then idle) Act engine to shorten the tail.
                copy_eng = nc.scalar if lc == NLC - 1 else nc.vector
                on = onorm_pool.tile([64, 512], bf16, tag="onorm", name="on")
                for h in range(2):
                    oacc = oaccs[2 * lc + h]
                    rr = small_pool.tile([1, 512], f32, tag="rr", name="rr")
                    nc.vector.reciprocal(rr, oacc[32:33, :])
                    rb = small_pool.tile([32, 512], f32, tag="rb", name="rb")
                    nc.gpsimd.partition_broadcast(rb, rr)
                    nc.vector.tensor_mul(on[32 * h:32 * (h + 1), :],
                                         oacc[0:32, :], rb)
                for jj in range(2):
                    po = ps_self.tile([128, 512], f32, tag="self",
                                      name="ps_self_t")
                    for j2 in range(2):
                        j = 2 * jj + j2
                        onj = on[:, 128 * j:128 * (j + 1)]
                        nc.tensor.matmul(po[:, 256 * j2:256 * (j2 + 1)],
                                         onj, wo_pair, start=True, stop=True)
                    osb = outsb_pool.tile([128, 512], f32, tag="outsb",
                                          name="osb")
                    if lc == NLC - 1:
                        nc.scalar.activation(osb, po,
                                             mybir.ActivationFunctionType.Copy)
                    else:
                        nc.vector.tensor_copy(osb, po)
                    r0 = LC * lc + 256 * jj
                    dst = out_d[r0:r0 + 256, :].rearrange(
                        "(j r) c -> r j c", j=2)
                    nc.sync.dma_start(dst, osb.rearrange("r (j c) -> r j c", j=2))

            # ---- software-pipelined emission ------------------------------
            # S(0)'s tp group k reads key tiles t=2k..2k+3 from KT chunk
            # lc=k//2, so interleaving with proj(lc) is program-order safe
            proj(0)
            S(0, 0, 3)
            proj(1)
            S(0, 3, 5)
            proj(2)
            S(0, 5, 7)
            proj(3)
            S(0, 7, 9)
            S(1)
            vbuild()
            P(0)
            S(2)
            P(1)
            S(3)
            O(0)
            P(2)
            S(4)
            P(3)
            S(5)
            O(1)
            P(4)
            S(6)
            P(5)
            S(7)
            O(2)
            P(6)
            P(7)
            O(3)
    nc.finalize()
    return nc


def _prep_inputs(query, in_proj_weight, in_proj_bias, in_proj_weight_self,
                 in_proj_bias_self, out_proj_weight, perm):
    """Per-core input maps (host-side transposes, permutation, scaling)."""
    scaling = np.float32(D ** -0.5)
    q_perm = np.asarray(query)[perm]          # (L, NB, E)

    Wq = np.asarray(in_proj_weight[0:E])
    Wk = np.asarray(in_proj_weight[E:2 * E])
    Wv = np.asarray(in_proj_weight[2 * E:3 * E])
    Wqs = np.asarray(in_proj_weight_self[0:E])
    Wks = np.asarray(in_proj_weight_self[E:2 * E])
    bq = np.asarray(in_proj_bias[0:E])
    bk = np.asarray(in_proj_bias[E:2 * E])
    bqs = np.asarray(in_proj_bias_self[0:E])
    bks = np.asarray(in_proj_bias_self[E:2 * E])
    WoT = np.ascontiguousarray(np.asarray(out_proj_weight).T)  # (E, E)

    xTs = [np.ascontiguousarray(q_perm[:, n, :].T).astype(bfloat16)
           for n in range(NB)]

    sgn = np.concatenate([-np.ones(32), np.ones(32),
                          -np.ones(32), np.ones(32)]).astype(np.float32)

    in_maps = []
    for c in range(NCORES):
        n = c // 4
        h0 = (2 * c) % H
        h1 = h0 + 1

        def hsl(W, h):
            return W[D * h:D * (h + 1)]

        wq_c = np.concatenate(
            [hsl(Wq, h0), hsl(Wqs, h0), hsl(Wq, h1), hsl(Wqs, h1)], 0) * scaling
        wk_c = np.concatenate(
            [hsl(Wk, h0), hsl(Wks, h0), hsl(Wk, h1), hsl(Wks, h1)], 0)
        wv_c = np.concatenate([hsl(Wv, h0), hsl(Wv, h1)], 0)
        bq_c = np.concatenate(
            [hsl(bq, h0), hsl(bqs, h0), hsl(bq, h1), hsl(bqs, h1)], 0) * scaling
        bk_c = np.concatenate(
            [hsl(bk, h0), hsl(bks, h0), hsl(bk, h1), hsl(bks, h1)], 0)
        wo_c = np.concatenate([WoT[D * h0:D * (h0 + 1)],
                               WoT[D * h1:D * (h1 + 1)]], 0)

        wqkv_c = np.concatenate([wq_c.T, wk_c.T, wv_c.T], axis=1)  # (E, 320)
        bias4_c = np.stack([bq_c, bk_c, sgn * bq_c, sgn], axis=1)  # (128, 4)
        wpack = np.zeros((128, 896), dtype=np.float32)
        wpack[:, 0:320] = wqkv_c[0:128]
        wpack[:, 320:640] = wqkv_c[128:256]
        wpack[0:64, 640:896] = wo_c                                # (64, 256)

        in_maps.append({
            "xT": xTs[n],
            "wpack": wpack.astype(bfloat16),
            "bias4": np.ascontiguousarray(bias4_c).astype(np.float32),
        })
    return in_maps


def _run(nc, in_maps, trace=False):
    from concourse.bass_utils import run_bass_kernel_spmd
    return run_bass_kernel_spmd(nc, in_maps, list(range(NCORES)), trace=trace)


def kernel(query, in_proj_weight, in_proj_bias, in_proj_weight_self,
           in_proj_bias_self, out_proj_weight, out_proj_bias,
           q_identities, k_identities, _trace=False, _return_br=False):
    ids = np.asarray(q_identities)
    perm, blocks = _block_structure(ids)

    key = ids.tobytes()
    if key not in _PROGRAM_CACHE:
        _PROGRAM_CACHE[key] = _build_program(_rects(blocks))
    nc = _PROGRAM_CACHE[key]

    in_maps = _prep_inputs(query, in_proj_weight, in_proj_bias,
                           in_proj_weight_self, in_proj_bias_self,
                           out_proj_weight, perm)
    br = _run(nc, in_maps, trace=_trace)

    # ---- unshard --------------------------------------------------------------
    # host bias: out_proj_bias + contribution of the v-bias through out_proj
    bias_total = (np.asarray(out_proj_bias)
                  + np.asarray(out_proj_weight) @ np.asarray(in_proj_bias)[2 * E:])
    out = np.zeros((L, NB, E), dtype=np.float32)
    for c in range(NCORES):
        n = c // 4
        out[:, n, :] += br.results[c]["out"]
    out += bias_total[None, None, :].astype(np.float32)
    # un-permute rows
    out_full = np.empty_like(out)
    out_full[perm] = out
    if _return_br:
        return out_full, br
    return out_full


# revision 55
# speedup vs baseline: 1.0296x; 1.0296x over previous
"""AgentAwareAttention Trainium2 kernel.

Full (unsharded) inputs -> full output.  Internally: 16 (batch, head) pairs
sharded 2-per-core across 8 NeuronCores; host pre-transposes operands and
sorts the sequence by agent id so the agent-equality mask becomes
block-diagonal.

Device-side structure (per core, heads h0/h1 of one batch):
  - projections:  QT = [q|qs] (bf16), QM = [-q|qs], KT = [k|ks]
  - main scores:  KT[k].T @ QT[q]           (32-contract, bf16)
  - delta scores: mask(KT)[k;ks].T @ QM[-q;qs]  adds (qs.ks - q.k) on the
    block-diagonal rects; masks are zero-padded KT column tiles built on the
    idle Pool engine (zeroed via KT*0 so the zeroing DEPENDS on KT and the
    scheduler cannot hoist every mask ahead of the first dep-ready copy).
  - exp on the Act engine is the bottleneck (~66us of [128,1024] tiles);
    PV uses ones-augmented V (rowsum for free); normalize + merged-head
    64-contract out-proj; j-pairs share one psum bank / osb / output DMA.

All four projection chunks are emitted BEFORE any score unit that reads
them (every S(u) reads all 16 KT key tiles -- interleaving proj(lc) behind
S(u) is a read-before-write race that CoreSim catches).  Emission is
software-pipelined over units u=(lc,h): S(u)=scores+exp, P(u)=PV,
O(lc)=normalize+outproj, ordered so the in-order PE stream never blocks
the Act engine's exp stream:
  p0 S0a p1 S0b p2 S0c p3 S0d S1 V P0 S2 P1 S3 O0 P2 S4 P3 S5 O1 P4 S6 P5 S7 O2 P6 P7 O3

Shapes (hardcoded): L=2048, N=2, E=256, H=8, D=32, N_AGENTS=16.
"""

import os

import numpy as np
from ml_dtypes import bfloat16


L = 2048
NB = 2        # batch
E = 256       # embed dim
H = 8         # heads
D = 32        # head dim
NCORES = 8
LC = 512      # l-chunk (moving-operand free dim)
NT = L // 128   # 16 s'-tiles of 128
NLC = L // LC   # 4 l-chunks

_PROGRAM_CACHE = {}


def _block_structure(ids):
    """Sort positions by agent id.  Returns perm and per-agent ranges in
    permuted space."""
    ids = np.asarray(ids)
    perm = np.argsort(ids, kind="stable")
    sids = ids[perm]
    bounds = [0]
    for i in range(1, len(sids)):
        if sids[i] != sids[i - 1]:
            bounds.append(i)
    bounds.append(len(sids))
    blocks = [(bounds[i], bounds[i + 1]) for i in range(len(bounds) - 1)]
    return perm, blocks


def _rects(blocks):
    """rects[(t, lc)] -> list of (r0, r1, c0, c1): the part of diagonal block
    (rows x cols, both = the block's range) that intersects s'-tile t
    (rows [128t,128t+128)) and l-chunk lc (cols [LC*lc, LC*lc+LC)), in
    tile-local coordinates."""
    rects = {}
    for (b0, b1) in blocks:
        for t in range(NT):
            r0 = max(b0, 128 * t)
            r1 = min(b1, 128 * t + 128)
            if r0 >= r1:
                continue
            for lc in range(NLC):
                c0 = max(b0, LC * lc)
                c1 = min(b1, LC * lc + LC)
                if c0 >= c1:
                    continue
                rects.setdefault((t, lc), []).append(
                    (r0 - 128 * t, r1 - 128 * t, c0 - LC * lc, c1 - LC * lc)
                )
    return rects


def _build_program(rects):
    import numpy as np
    import concourse.mybir as mybir
    import concourse.tile as tile
    from concourse import bacc

    f32 = mybir.dt.float32
    bf16 = mybir.dt.bfloat16
    add = mybir.AluOpType.add
    mult = mybir.AluOpType.mult

    # masks needed: (t, r0, r1) -> arena index; full-tile rects use KT directly
    mask_idx = {}
    for (t, lc), rl in sorted(rects.items()):
        for (r0, r1, c0, c1) in rl:
            if (r0, r1) == (0, 128):
                continue
            key = (t, r0, r1)
            if key not in mask_idx:
                mask_idx[key] = len(mask_idx)
    n_masks = max(1, len(mask_idx))

    nc = bacc.Bacc(None)

    xT_d = nc.declare_dram_parameter("xT", [E, L], bf16, isOutput=False)
    # wpack: [wqkv half0 (320) | wqkv half1 (320) | wo (256) | xt0 lc0 (512)
    #         | xt1 lc0 (512)] -- lc0's xt rides the same DMA as the weights
    wpack_d = nc.declare_dram_parameter("wpack", [128, 1920], bf16, isOutput=False)
    bias4_d = nc.declare_dram_parameter("bias4", [128, 4], f32, isOutput=False)
    out_d = nc.declare_dram_parameter("out", [L, E], f32, isOutput=True)

    with tile.TileContext(nc) as tc:
        with (
            tc.tile_pool(name="consts", bufs=1) as consts,
            tc.tile_pool(name="pslab", bufs=3) as pslab_pool,
            tc.tile_pool(name="onorm", bufs=4) as onorm_pool,
            tc.tile_pool(name="small", bufs=4) as small_pool,
            tc.tile_pool(name="outsb", bufs=3) as outsb_pool,
            tc.tile_pool(name="ps_score", bufs=2, space="PSUM") as ps_score,
            tc.tile_pool(name="ps_self", bufs=2, space="PSUM") as ps_self,
            tc.tile_pool(name="ps_oacc", bufs=2, space="PSUM") as ps_oacc,
        ):
            # ---- constant loads -------------------------------------------
            # HWDGE serializes DMA issue at ~625ns each, so everything
            # constant rides ONE packed DMA; xt is chunked so proj(0) can
            # start as soon as its first 512 columns land.
            xt = [consts.tile([128, L], bf16, tag=f"xt{i}", name=f"xt{i}")
                  for i in range(2)]
            wpack = consts.tile([128, 1920], bf16, tag="wpack", name="wpack")
            bias4 = consts.tile([128, 4], f32, tag="bias4", name="bias4")
            wq = [wpack[:, 320 * i:320 * i + 128] for i in range(2)]
            wk = [wpack[:, 320 * i + 128:320 * i + 256] for i in range(2)]
            wv = [wpack[:, 320 * i + 256:320 * i + 320] for i in range(2)]
            bq_t = bias4[:, 0:1]
            bk_t = bias4[:, 1:2]
            bqm_t = bias4[:, 2:3]
            sg_t = bias4[:, 3:4]
            wo_pair = wpack[0:64, 640:896]
            xt_lc0 = [wpack[:, 896 + 512 * i:896 + 512 * (i + 1)]
                      for i in range(2)]

            def xs(i, a, b):
                """xt[i] columns [a,b) -- lc0 columns live inside wpack."""
                if b <= LC:
                    return xt_lc0[i][:, a:b]
                return xt[i][:, a:b]

            nc.sync.dma_start(wpack, wpack_d[:, :])
            nc.sync.dma_start(bias4, bias4_d[:, :])
            for i in range(2):
                nc.sync.dma_start(xt[i][:, LC:2 * LC],
                                  xT_d[128 * i:128 * (i + 1), LC:2 * LC])
            for i in range(2):
                nc.sync.dma_start(xt[i][:, 2 * LC:],
                                  xT_d[128 * i:128 * (i + 1), 2 * LC:])

            QT = consts.tile([128, L], bf16, tag="QT", name="QT")
            QM = consts.tile([128, L], bf16, tag="QM", name="QM")
            KT = consts.tile([128, L], bf16, tag="KT", name="KT")
            marena = consts.tile([128, n_masks, 128], bf16, tag="marena",
                                 name="marena")
            v_sb = consts.tile([128, NT, 66], bf16, tag="vsb", name="v_sb")

            # PE warm-up: TRN2 pstate ramps (0.65 -> 1.2 -> 2.4 GHz) key off
            # continuous PE activity; a few throwaway matmuls on a memset
            # tile keep the projection matmuls off the cold clock.
            dummy = consts.tile([128, 512], bf16, tag="dummy", name="dummy")
            dume = consts.tile([1, 1], bf16, tag="dume", name="dume")
            nc.gpsimd.memset(dummy, 0.0)
            nc.gpsimd.memset(dume, 0.0)
            nc.gpsimd.memset(v_sb, 1.0)
            # hoist the Exp act-table load off the critical path
            nc.scalar.activation(dume, dummy[0:1, 0:1],
                                 mybir.ActivationFunctionType.Exp)
            for w in range(2):
                pdum = ps_self.tile([128, 512], f32, tag="self",
                                    name="ps_self_t")
                nc.tensor.matmul(pdum, dummy[:, 0:128], dummy,
                                 start=True, stop=True)

            def delta_lhs(t, r0, r1):
                if (r0, r1) == (0, 128):
                    return KT[:, 128 * t:128 * (t + 1)]
                m = mask_idx[(t, r0, r1)]
                return marena[:, m, :]

            def proj(lc):
                sl = slice(LC * lc, LC * (lc + 1))
                pk = ps_self.tile([128, 512], f32, tag="self", name="ps_self_t")
                nc.tensor.matmul(pk, wk[0], xs(0, sl.start, sl.stop),
                                 start=True, stop=False)
                nc.tensor.matmul(pk, wk[1], xs(1, sl.start, sl.stop),
                                 start=False, stop=True)
                nc.vector.tensor_scalar(
                    out=KT[:, sl], in0=pk, scalar1=bk_t, scalar2=None, op0=add)
                # mask tiles sourced from this KT chunk (idle Pool engine).
                # zeroing via KT*0 (not memset) so it DEPENDS on this KT
                # chunk -- otherwise the scheduler hoists every no-dep memset
                # ahead of the first dep-ready copy and starves lc0's masks.
                for (t, r0, r1), m in sorted(mask_idx.items(), key=lambda kv: kv[1]):
                    if t // 4 == lc:
                        nc.gpsimd.tensor_scalar(
                            out=marena[:, m, :],
                            in0=KT[:, 128 * t:128 * (t + 1)],
                            scalar1=0.0, scalar2=None, op0=mult)
                        nc.gpsimd.tensor_copy(
                            marena[:, m, r0:r1],
                            KT[:, 128 * t + r0:128 * t + r1])
                pq = ps_self.tile([128, 512], f32, tag="self", name="ps_self_t")
                nc.tensor.matmul(pq, wq[0], xs(0, sl.start, sl.stop),
                                 start=True, stop=False)
                nc.tensor.matmul(pq, wq[1], xs(1, sl.start, sl.stop),
                                 start=False, stop=True)
                nc.vector.tensor_scalar(
                    out=QT[:, sl], in0=pq, scalar1=bq_t, scalar2=None, op0=add)
                if lc == 0:
                    # Act idles pre-exp; DVE's scheduler otherwise orders KT1
                    # ahead of QM0 and delays the first delta matmuls
                    nc.scalar.activation(
                        QM[:, sl], pq, mybir.ActivationFunctionType.Identity,
                        bias=bqm_t, scale=sg_t)
                else:
                    nc.vector.tensor_scalar(
                        out=QM[:, sl], in0=pq, scalar1=sg_t, scalar2=bqm_t,
                        op0=mult, op1=add)

            def vbuild():
                for t in range(NT):
                    ts = slice(128 * t, 128 * (t + 1))
                    pv = ps_self.tile([128, 512], f32, tag="self",
                                      name="ps_self_t")
                    nc.tensor.matmul(pv[:, 0:64], xs(0, ts.start, ts.stop),
                                     wv[0], start=True, stop=False)
                    nc.tensor.matmul(pv[:, 0:64], xs(1, ts.start, ts.stop),
                                     wv[1], start=False, stop=True)
                    nc.vector.tensor_copy(v_sb[:, t, 0:32], pv[:, 0:32])
                    nc.vector.tensor_copy(v_sb[:, t, 33:65], pv[:, 32:64])

            pslabs = {}
            oaccs = {}

            # score groups: contiguous t-runs of 3 (then the leftover t15)
            # -> 6 exps of width <=1536 per (lc,h) instead of 8x1024,
            # amortizing the Act engine's fixed per-instruction access cost
            SGROUPS = [[0, 1], [2, 3], [4, 5], [6, 7], [8, 9], [10, 11],
                       [12, 13], [14, 15]]
            # unit 0 leads with a narrow [t0] group: exp0 fires right after
            # QM0 lands instead of waiting for a full 1024-wide group
            SGROUPS0 = [[0], [1], [2, 3], [4, 5], [6, 7], [8, 9], [10, 11],
                        [12, 13], [14, 15]]
            # units 1-6: two groups each on DVE; final unit alternates so
            # Act and DVE drain unit 7's exps in parallel (shorter tail)
            DVE_EXP_GROUPS = {2, 5}
            DVE_EXP_GROUPS_LAST = {1, 3, 5, 7}

            def S(u, g0=0, g1=None):
                lc, h = u // 2, u % 2
                groups = SGROUPS0 if u == 0 else SGROUPS
                if g1 is None:
                    g1 = len(groups)
                lsl = slice(LC * lc, LC * (lc + 1))
                qb = 64 * h
                if g0 == 0:
                    pslab = pslab_pool.tile([128, NT * 512], bf16, tag="pslab",
                                            name="pslab")
                    pslabs[u] = pslab
                else:
                    pslab = pslabs[u]
                for g in range(g0, g1):
                    ts = groups[g]
                    w = 512 * len(ts)
                    ps2 = ps_score.tile([128, 1024], f32, tag="score",
                                        name="ps2")
                    for k, t in enumerate(ts):
                        tsl = slice(128 * t, 128 * (t + 1))
                        o = 512 * k
                        rl = rects.get((t, lc), [])
                        nc.tensor.matmul(
                            ps2[:, o:o + 512],
                            KT[qb:qb + 32, tsl], QT[qb:qb + 32, lsl],
                            start=True, stop=(not rl), tile_position=(qb, 0))
                        for i, (r0, r1, c0, c1) in enumerate(rl):
                            mk = delta_lhs(t, r0, r1)
                            nc.tensor.matmul(
                                ps2[:, o + c0:o + c1],
                                mk[qb:qb + 64, :],
                                QM[qb:qb + 64, LC * lc + c0:LC * lc + c1],
                                start=False, stop=(i == len(rl) - 1),
                                tile_position=(qb, 0))
                    dst = pslab[:, 512 * ts[0]:512 * ts[0] + w]
                    dve_g = (DVE_EXP_GROUPS_LAST if u == NLC * 2 - 1
                             else DVE_EXP_GROUPS)
                    if g in dve_g and u >= 1:
                        # Schraudolph bit-trick exp on the otherwise-idle DVE:
                        # bf16 bits = round(x*128/ln2 + (16256-C)); softmax
                        # normalization cancels the correlated approx error
                        # (measured end-to-end ~2e-3 even at 100% offload)
                        nc.vector.tensor_scalar(
                            out=dst.bitcast(mybir.dt.int16), in0=ps2[:, 0:w],
                            scalar1=float(128.0 / np.log(2.0)),
                            scalar2=16256.0 - 7.42,
                            op0=mult, op1=add)
                    else:
                        nc.scalar.activation(
                            dst, ps2[:, 0:w],
                            mybir.ActivationFunctionType.Exp)

            def P(u):
                lc, h = u // 2, u % 2
                pslab = pslabs[u]
                oacc = ps_oacc.tile([33, 512], f32, tag="oacc", name="oacc")
                oaccs[u] = oacc
                for t in range(NT):
                    nc.tensor.matmul(
                        oacc, v_sb[:, t, 33 * h:33 * h + 33],
                        pslab[:, 512 * t:512 * (t + 1)],
                        start=(t == 0), stop=(t == NT - 1))

            def O(lc):
                # normalize both heads into one [64, 512] tile, then a single
                # 64-contract out-proj matmul per j-tile; j-pairs share one
                # psum bank / osb / output DMA.  The last chunk's copies ride
                # the (by then idle) Act engine to shorten the tail.
                copy_eng = nc.scalar if lc == NLC - 1 else nc.vector
                on = onorm_pool.tile([64, 512], bf16, tag="onorm", name="on")
                for h in range(2):
                    oacc = oaccs[2 * lc + h]
                    rr = small_pool.tile([1, 512], f32, tag="rr", name="rr")
                    nc.vector.reciprocal(rr, oacc[32:33, :])
                    rb = small_pool.tile([32, 512], f32, tag="rb", name="rb")
                    nc.gpsimd.partition_broadcast(rb, rr)
                    nc.vector.tensor_mul(on[32 * h:32 * (h + 1), :],
                                         oacc[0:32, :], rb)
                for jj in range(2):
                    po = ps_self.tile([128, 512], f32, tag="self",
                                      name="ps_self_t")
                    for j2 in range(2):
                        j = 2 * jj + j2
                        onj = on[:, 128 * j:128 * (j + 1)]
                        nc.tensor.matmul(po[:, 256 * j2:256 * (j2 + 1)],
                                         onj, wo_pair, start=True, stop=True)
                    osb = outsb_pool.tile([128, 512], f32, tag="outsb",
                                          name="osb")
                    if lc == NLC - 1:
                        nc.scalar.activation(osb, po,
                                             mybir.ActivationFunctionType.Copy)
                    else:
                        nc.vector.tensor_copy(osb, po)
                    r0 = LC * lc + 256 * jj
                    dst = out_d[r0:r0 + 256, :].rearrange(
                        "(j r) c -> r j c", j=2)
                    nc.sync.dma_start(dst, osb.rearrange("r (j c) -> r j c", j=2))

            # ---- software-pipelined emission ------------------------------
            # S(0)'s tp group k reads key tiles t=2k..2k+3 from KT chunk
            # lc=k//2, so interleaving with proj(lc) is program-order safe
            proj(0)
            S(0, 0, 3)
            proj(1)
            S(0, 3, 5)
            proj(2)
            S(0, 5, 7)
            proj(3)
            S(0, 7, 9)
            S(1)
            vbuild()
            P(0)
            S(2)
            P(1)
            S(3)
            O(0)
            P(2)
            S(4)
            P(3)
            S(5)
            O(1)
            P(4)
            S(6)
            P(5)
            S(7)
            O(2)
            P(6)
            P(7)
            O(3)
    nc.finalize()
    return nc


def _prep_inputs(query, in_proj_weight, in_proj_bias, in_proj_weight_self,
                 in_proj_bias_self, out_proj_weight, perm):
    """Per-core input maps (host-side transposes, permutation, scaling)."""
    scaling = np.float32(D ** -0.5)
    q_perm = np.asarray(query)[perm]          # (L, NB, E)

    Wq = np.asarray(in_proj_weight[0:E])
    Wk = np.asarray(in_proj_weight[E:2 * E])
    Wv = np.asarray(in_proj_weight[2 * E:3 * E])
    Wqs = np.asarray(in_proj_weight_self[0:E])
    Wks = np.asarray(in_proj_weight_self[E:2 * E])
    bq = np.asarray(in_proj_bias[0:E])
    bk = np.asarray(in_proj_bias[E:2 * E])
    bqs = np.asarray(in_proj_bias_self[0:E])
    bks = np.asarray(in_proj_bias_self[E:2 * E])
    WoT = np.ascontiguousarray(np.asarray(out_proj_weight).T)  # (E, E)

    xTs = [np.ascontiguousarray(q_perm[:, n, :].T).astype(bfloat16)
           for n in range(NB)]

    sgn = np.concatenate([-np.ones(32), np.ones(32),
                          -np.ones(32), np.ones(32)]).astype(np.float32)

    in_maps = []
    for c in range(NCORES):
        n = c // 4
        h0 = (2 * c) % H
        h1 = h0 + 1

        def hsl(W, h):
            return W[D * h:D * (h + 1)]

        wq_c = np.concatenate(
            [hsl(Wq, h0), hsl(Wqs, h0), hsl(Wq, h1), hsl(Wqs, h1)], 0) * scaling
        wk_c = np.concatenate(
            [hsl(Wk, h0), hsl(Wks, h0), hsl(Wk, h1), hsl(Wks, h1)], 0)
        wv_c = np.concatenate([hsl(Wv, h0), hsl(Wv, h1)], 0)
        bq_c = np.concatenate(
            [hsl(bq, h0), hsl(bqs, h0), hsl(bq, h1), hsl(bqs, h1)], 0) * scaling
        bk_c = np.concatenate(
            [hsl(bk, h0), hsl(bks, h0), hsl(bk, h1), hsl(bks, h1)], 0)
        wo_c = np.concatenate([WoT[D * h0:D * (h0 + 1)],
                               WoT[D * h1:D * (h1 + 1)]], 0)

        wqkv_c = np.concatenate([wq_c.T, wk_c.T, wv_c.T], axis=1)  # (E, 320)
        bias4_c = np.stack([bq_c, bk_c, sgn * bq_c, sgn], axis=1)  # (128, 4)
        wpack = np.zeros((128, 1920), dtype=np.float32)
        wpack[:, 0:320] = wqkv_c[0:128]
        wpack[:, 320:640] = wqkv_c[128:256]
        wpack[0:64, 640:896] = wo_c                                # (64, 256)
        wpack[:, 896:1408] = xTs[n][0:128, 0:512].astype(np.float32)
        wpack[:, 1408:1920] = xTs[n][128:256, 0:512].astype(np.float32)

        in_maps.append({
            "xT": xTs[n],
            "wpack": wpack.astype(bfloat16),
            "bias4": np.ascontiguousarray(bias4_c).astype(np.float32),
        })
    return in_maps


def _run(nc, in_maps, trace=False):
    from concourse.bass_utils import run_bass_kernel_spmd
    return run_bass_kernel_spmd(nc, in_maps, list(range(NCORES)), trace=trace)


def kernel(query, in_proj_weight, in_proj_bias, in_proj_weight_self,
           in_proj_bias_self, out_proj_weight, out_proj_bias,
           q_identities, k_identities, _trace=False, _return_br=False):
    ids = np.asarray(q_identities)
    perm, blocks = _block_structure(ids)

    key = ids.tobytes()
    if key not in _PROGRAM_CACHE:
        _PROGRAM_CACHE[key] = _build_program(_rects(blocks))
    nc = _PROGRAM_CACHE[key]

    in_maps = _prep_inputs(query, in_proj_weight, in_proj_bias,
                           in_proj_weight_self, in_proj_bias_self,
                           out_proj_weight, perm)
    br = _run(nc, in_maps, trace=_trace)

    # ---- unshard --------------------------------------------------------------
    # host bias: out_proj_bias + contribution of the v-bias through out_proj
    bias_total = (np.asarray(out_proj_bias)
                  + np.asarray(out_proj_weight) @ np.asarray(in_proj_bias)[2 * E:])
    out = np.zeros((L, NB, E), dtype=np.float32)
    for c in range(NCORES):
        n = c // 4
        out[:, n, :] += br.results[c]["out"]
    out += bias_total[None, None, :].astype(np.float32)
    # un-permute rows
    out_full = np.empty_like(out)
    out_full[perm] = out
    if _return_br:
        return out_full, br
    return out_full
